# revision 14
# baseline (speedup 1.0000x reference)
"""AGNNConv (single-head attention message passing) on 8 TRN2 NeuronCores.

Reference computation (N=100000 nodes, fixed degree 16, D=64):
    X_prime = X @ W                                  # [N, 64]
    e[n,k]  = <X_prime[n], X_prime[ci[n,k]]> * s     # s = attention_w[0,0]
    out[n]  = sum_k e[n,k] * X_prime[ci[n,k]]        # [N, 64]

Sharding: nodes split 12500/core across 8 cores. Each core computes its
X_prime shard (and a pre-scaled copy via W*s), AllGathers the bf16 table,
then gathers its nodes' 16 neighbor rows per 128-node tile with an
indirect DMA and does the dot/weight/aggregate on the vector engine.
"""

import math
import sys

import numpy as np

if "/opt/trn_rl_repo" not in sys.path:
    sys.path.insert(0, "/opt/trn_rl_repo")

N_NODES = 100000
DEG = 16
D = 64
CORES = 8
NPC = N_NODES // CORES  # 12500
P = 128
NTILES = (NPC + P - 1) // P  # 98
NPAD = NTILES * P  # 12544


def build_nc(n_nodes=N_NODES, npc=NPC, deg=DEG, d=D, cores=CORES, lowering=False):
    from concourse import bacc, bass, mybir, tile
    from concourse import tile_sem_assignment as _tsa

    # One SWDGE bookkeeping lane: the 1568 indirect gathers then need ~1
    # semaphore wait per 16-gather group instead of ~8 (the rotating lanes
    # each re-emit), shaving per-instruction sequencer overhead.
    _tsa.NUM_SWDGE_GLOBAL_SEMS = 1

    ntiles = (npc + P - 1) // P
    npad = ntiles * P

    f32 = mybir.dt.float32
    bf16 = mybir.dt.bfloat16
    i32 = mybir.dt.int32

    nc = bacc.Bacc(
        "TRN2", target_bir_lowering=lowering, debug=False, num_devices=cores
    )

    # xT carries [X_shard.T | W | W*s] so the matmuls depend on ONE input DMA
    # (the Matmult LdWeights slot only fits a single semaphore wait).
    xT = nc.declare_dram_parameter("xT", [d, npad + 2 * d], f32, isOutput=False)
    ci = nc.declare_dram_parameter("ci", [npad, deg], i32, isOutput=False)
    out_ext = nc.declare_dram_parameter("out", [npad, d], f32, isOutput=True)

    # Padded to npad rows so ONE DMA fills cc_in (the collective-trigger
    # ISA struct only fits a single semaphore wait). Neighbor indices are
    # host-remapped to this padded row numbering.
    cc_in = nc.dram_tensor("cc_in", [npad, d], bf16)
    cc_out = nc.dram_tensor("cc_out", [cores * npad, d], bf16, addr_space="Shared")

    with tile.TileContext(nc) as tc:
        with (
            tc.tile_pool(name="const", bufs=1) as cpool,
            tc.tile_pool(name="psum", bufs=4, space="PSUM") as psum,
            tc.tile_pool(name="g", bufs=8) as gpool,
            tc.tile_pool(name="prod", bufs=2) as ppool,
            tc.tile_pool(name="q", bufs=2) as qpool,
            tc.tile_pool(name="e", bufs=3) as epool,
            tc.tile_pool(name="o", bufs=3) as opool,
        ):
            xT_sb = cpool.tile([d, npad + 2 * d], f32, tag="xT_sb")
            xp_bf = cpool.tile([P, ntiles * d], bf16, tag="xp_bf")
            sxp_bf = cpool.tile([P, ntiles * d], bf16, tag="sxp_bf")
            idx_sb = cpool.tile([P, ntiles * deg], i32, tag="idx_sb")

            nc.sync.dma_start(out=xT_sb[:, :], in_=xT[:, :])
            w_sb = xT_sb[:, npad : npad + d]
            w2_sb = xT_sb[:, npad + d : npad + 2 * d]
            nc.sync.dma_start(
                out=idx_sb[:, :].rearrange("p (t k) -> p t k", t=ntiles),
                in_=ci[:, :].rearrange("(t p) k -> p t k", p=P),
            )

            # X_prime shard (bf16) and pre-scaled X_prime*s shard.
            for t in range(ntiles):
                ps1 = psum.tile([P, d], f32, tag="ps1")
                nc.tensor.matmul(
                    ps1[:, :],
                    xT_sb[:, t * P : (t + 1) * P],
                    w_sb,
                    start=True,
                    stop=True,
                )
                nc.vector.tensor_copy(out=xp_bf[:, t * d : (t + 1) * d], in_=ps1[:, :])
                ps2 = psum.tile([P, d], f32, tag="ps2")
                nc.tensor.matmul(
                    ps2[:, :],
                    xT_sb[:, t * P : (t + 1) * P],
                    w2_sb,
                    start=True,
                    stop=True,
                )
                nc.vector.tensor_copy(out=sxp_bf[:, t * d : (t + 1) * d], in_=ps2[:, :])

            # Shard -> internal DRAM -> AllGather full bf16 table.
            nc.sync.dma_start(
                out=cc_in[:, :].rearrange("(t p) f -> p t f", p=P),
                in_=xp_bf[:, :].rearrange("p (t f) -> p t f", t=ntiles),
            )
            nc.gpsimd.collective_compute(
                "AllGather",
                mybir.AluOpType.bypass,
                replica_groups=[list(range(cores))],
                ins=[cc_in.ap()],
                outs=[cc_out.ap()],
            )

            # The SWDGE queue descriptor fits only ONE semaphore wait, but the
            # first gather depends on both the collective (cc_out) and the idx
            # DMA. Absorb each wait into the SWDGE proc with a tiny DMA first.
            scr = cpool.tile([1, d], bf16, tag="scr")
            scr2 = cpool.tile([1, deg], i32, tag="scr2")
            d1 = nc.gpsimd.dma_start(out=scr[:, :], in_=cc_out[0:1, :])
            d2 = nc.gpsimd.dma_start(out=scr2[:, :], in_=idx_sb[0:1, 0:deg])

            # Gather + edge compute, one 128-node tile at a time.
            from concourse.tile import add_dep_helper

            for t in range(ntiles):
                G = gpool.tile([P, deg * d], bf16, tag="G")
                for k in range(deg):
                    # HW DGE consumes ONE offset per partition per instruction:
                    # each descriptor gathers a contiguous row into this
                    # neighbor's slot of G.
                    gi = nc.gpsimd.indirect_dma_start(
                        out=G[:, k * d : (k + 1) * d],
                        out_offset=None,
                        in_=cc_out[:, :],
                        in_offset=bass.IndirectOffsetOnAxis(
                            ap=idx_sb[:, t * deg + k : t * deg + k + 1], axis=0
                        ),
                    )
                    if t < 8:
                        add_dep_helper(gi.ins, d1.ins, False, "swdge wait absorber")
                        add_dep_helper(gi.ins, d2.ins, False, "swdge wait absorber")
                Gv = G[:, :].rearrange("p (k f) -> p k f", k=deg)
                Pt = ppool.tile([P, deg * d], bf16, tag="Pt")
                nc.vector.tensor_tensor(
                    out=Pt[:, :].rearrange("p (k f) -> p k f", k=deg),
                    in0=Gv,
                    in1=sxp_bf[:, t * d : (t + 1) * d]
                    .unsqueeze(1)
                    .broadcast_to([P, deg, d]),
                    op=mybir.AluOpType.mult,
                )
                e = epool.tile([P, deg], bf16, tag="e")
                with nc.allow_low_precision(reason="bf16 edge attn within tolerance"):
                    nc.vector.tensor_reduce(
                        out=e[:, :],
                        in_=Pt[:, :].rearrange("p (k f) -> p k f", k=deg),
                        axis=mybir.AxisListType.X,
                        op=mybir.AluOpType.add,
                    )
                Qt = qpool.tile([P, deg * d], bf16, tag="Qt")
                nc.vector.tensor_tensor(
                    out=Qt[:, :].rearrange("p (k f) -> p k f", k=deg),
                    in0=Gv,
                    in1=e[:, :].unsqueeze(2).broadcast_to([P, deg, d]),
                    op=mybir.AluOpType.mult,
                )
                o = opool.tile([P, d], f32, tag="o")
                nc.vector.tensor_reduce(
                    out=o[:, :],
                    in_=Qt[:, :].rearrange("p (k f) -> p f k", k=deg),
                    axis=mybir.AxisListType.X,
                    op=mybir.AluOpType.add,
                )
                nc.sync.dma_start(out=out_ext[t * P : (t + 1) * P, :], in_=o[:, :])

    nc.compile()
    return nc


def make_in_maps(X, weights, attention_w, column_index, n_nodes=N_NODES, cores=CORES):
    npc = n_nodes // cores
    ntiles = (npc + P - 1) // P
    npad = ntiles * P
    s = float(np.asarray(attention_w).reshape(-1)[0])
    w = np.asarray(weights, dtype=np.float32)
    ci_all = np.asarray(column_index, dtype=np.int32).reshape(n_nodes, DEG)
    in_maps = []
    for c in range(cores):
        r0, r1 = c * npc, (c + 1) * npc
        xT = np.zeros((D, npad + 2 * D), dtype=np.float32)
        xT[:, :npc] = np.asarray(X[r0:r1], dtype=np.float32).T
        xT[:, npad : npad + D] = w
        xT[:, npad + D : npad + 2 * D] = w * s
        ci = np.zeros((npad, DEG), dtype=np.int32)
        ci_shard = ci_all[r0:r1]
        # remap node id -> row in the npad-padded AllGather table
        ci[:npc] = (ci_shard // npc) * npad + (ci_shard % npc)
        in_maps.append(
            {
                "xT": np.ascontiguousarray(xT),
                "ci": np.ascontiguousarray(ci),
            }
        )
    return in_maps


_NC_CACHE = {}


def _get_nc():
    key = (N_NODES, NPC)
    if key not in _NC_CACHE:
        _NC_CACHE[key] = build_nc()
    return _NC_CACHE[key]


def run(X, weights, attention_w, column_index, trace=False, **trace_kwargs):
    from concourse import bass_utils

    nc = _get_nc()
    in_maps = make_in_maps(X, weights, attention_w, column_index)
    res = bass_utils.run_bass_kernel_spmd(
        nc, in_maps, core_ids=list(range(CORES)), trace=trace, **trace_kwargs
    )
    outs = [np.asarray(res.results[c]["out"][:NPC]) for c in range(CORES)]
    return np.concatenate(outs, axis=0).astype(np.float32), res


def kernel(
    X,
    weights,
    attention_w,
    row_pointers,
    column_index,
    blockPartition,
    edgeToColumn,
    edgeToRow,
    **_unused,
):
    out, _ = run(X, weights, attention_w, column_index)
    return out


# revision 15
# speedup vs baseline: 1.8291x; 1.8291x over previous
"""AGNNConv (single-head attention message passing) on 8 TRN2 NeuronCores.

Reference computation (N=100000 nodes, fixed degree 16, D=64):
    X_prime = X @ W                                  # [N, 64]
    e[n,k]  = <X_prime[n], X_prime[ci[n,k]]> * s     # s = attention_w[0,0]
    out[n]  = sum_k e[n,k] * X_prime[ci[n,k]]        # [N, 64]

Sharding: nodes split 12500/core across 8 cores. Each core computes its
X_prime shard (and a pre-scaled copy via W*s), AllGathers the bf16 table,
then gathers its nodes' 16 neighbor rows per 128-node tile with an
indirect DMA and does the dot/weight/aggregate on the vector engine.
"""

import math
import sys

import numpy as np

if "/opt/trn_rl_repo" not in sys.path:
    sys.path.insert(0, "/opt/trn_rl_repo")

N_NODES = 100000
DEG = 16
D = 64
CORES = 8
NPC = N_NODES // CORES  # 12500
P = 128
NTILES = (NPC + P - 1) // P  # 98
NPAD = NTILES * P  # 12544


def build_nc(n_nodes=N_NODES, npc=NPC, deg=DEG, d=D, cores=CORES, lowering=False):
    from concourse import bacc, bass, mybir, tile

    ntiles = (npc + P - 1) // P
    npad = ntiles * P

    f32 = mybir.dt.float32
    bf16 = mybir.dt.bfloat16
    i32 = mybir.dt.int32

    nc = bacc.Bacc(
        "TRN2", target_bir_lowering=lowering, debug=False, num_devices=cores
    )

    # xT carries [X_shard.T | W | W*s] so the matmuls depend on ONE input DMA
    # (the Matmult LdWeights slot only fits a single semaphore wait).
    xT = nc.declare_dram_parameter("xT", [d, npad + 2 * d], f32, isOutput=False)
    ci = nc.declare_dram_parameter("ci", [npad, deg], i32, isOutput=False)
    out_ext = nc.declare_dram_parameter("out", [npad, d], f32, isOutput=True)

    # Padded to npad rows so ONE DMA fills cc_in (the collective-trigger
    # ISA struct only fits a single semaphore wait). Neighbor indices are
    # host-remapped to this padded row numbering.
    cc_in = nc.dram_tensor("cc_in", [npad, d], bf16)
    cc_out = nc.dram_tensor("cc_out", [cores * npad, d], bf16, addr_space="Shared")

    with tile.TileContext(nc) as tc:
        with (
            tc.tile_pool(name="const", bufs=1) as cpool,
            tc.tile_pool(name="psum", bufs=4, space="PSUM") as psum,
            tc.tile_pool(name="g", bufs=8) as gpool,
            tc.tile_pool(name="prod", bufs=2) as ppool,
            tc.tile_pool(name="q", bufs=2) as qpool,
            tc.tile_pool(name="e", bufs=3) as epool,
            tc.tile_pool(name="o", bufs=3) as opool,
        ):
            xT_sb = cpool.tile([d, npad + 2 * d], f32, tag="xT_sb")
            xp_bf = cpool.tile([P, ntiles * d], bf16, tag="xp_bf")
            sxp_bf = cpool.tile([P, ntiles * d], bf16, tag="sxp_bf")
            idx_sb = cpool.tile([P, ntiles * deg], i32, tag="idx_sb")

            nc.sync.dma_start(out=xT_sb[:, :], in_=xT[:, :])
            w_sb = xT_sb[:, npad : npad + d]
            w2_sb = xT_sb[:, npad + d : npad + 2 * d]
            nc.sync.dma_start(
                out=idx_sb[:, :].rearrange("p (t k) -> p t k", t=ntiles),
                in_=ci[:, :].rearrange("(t p) k -> p t k", p=P),
            )

            # X_prime shard (bf16) and pre-scaled X_prime*s shard.
            for t in range(ntiles):
                ps1 = psum.tile([P, d], f32, tag="ps1")
                nc.tensor.matmul(
                    ps1[:, :],
                    xT_sb[:, t * P : (t + 1) * P],
                    w_sb,
                    start=True,
                    stop=True,
                )
                nc.vector.tensor_copy(out=xp_bf[:, t * d : (t + 1) * d], in_=ps1[:, :])
                ps2 = psum.tile([P, d], f32, tag="ps2")
                nc.tensor.matmul(
                    ps2[:, :],
                    xT_sb[:, t * P : (t + 1) * P],
                    w2_sb,
                    start=True,
                    stop=True,
                )
                nc.vector.tensor_copy(out=sxp_bf[:, t * d : (t + 1) * d], in_=ps2[:, :])

            # Shard -> internal DRAM -> AllGather full bf16 table.
            nc.sync.dma_start(
                out=cc_in[:, :].rearrange("(t p) f -> p t f", p=P),
                in_=xp_bf[:, :].rearrange("p (t f) -> p t f", t=ntiles),
            )
            nc.gpsimd.collective_compute(
                "AllGather",
                mybir.AluOpType.bypass,
                replica_groups=[list(range(cores))],
                ins=[cc_in.ap()],
                outs=[cc_out.ap()],
            )

            # The SWDGE queue descriptor fits only ONE semaphore wait, but the
            # first gather depends on both the collective (cc_out) and the idx
            # DMA. Absorb each wait into the SWDGE proc with a tiny DMA first.
            scr = cpool.tile([1, d], bf16, tag="scr")
            scr2 = cpool.tile([1, deg], i32, tag="scr2")
            d1 = nc.gpsimd.dma_start(out=scr[:, :], in_=cc_out[0:1, :])
            d2 = nc.gpsimd.dma_start(out=scr2[:, :], in_=idx_sb[0:1, 0:deg])

            # Gather + edge compute, one 128-node tile at a time.
            from concourse.tile import add_dep_helper

            for t in range(ntiles):
                G = gpool.tile([P, deg * d], bf16, tag="G")
                for k in range(deg):
                    # HW DGE consumes ONE offset per partition per instruction:
                    # each descriptor gathers a contiguous row into this
                    # neighbor's slot of G.
                    gi = nc.gpsimd.indirect_dma_start(
                        out=G[:, k * d : (k + 1) * d],
                        out_offset=None,
                        in_=cc_out[:, :],
                        in_offset=bass.IndirectOffsetOnAxis(
                            ap=idx_sb[:, t * deg + k : t * deg + k + 1], axis=0
                        ),
                    )
                    if t < 8:
                        add_dep_helper(gi.ins, d1.ins, False, "swdge wait absorber")
                        add_dep_helper(gi.ins, d2.ins, False, "swdge wait absorber")
                Gv = G[:, :].rearrange("p (k f) -> p k f", k=deg)
                Pt = ppool.tile([P, deg * d], bf16, tag="Pt")
                nc.vector.tensor_tensor(
                    out=Pt[:, :].rearrange("p (k f) -> p k f", k=deg),
                    in0=Gv,
                    in1=sxp_bf[:, t * d : (t + 1) * d]
                    .unsqueeze(1)
                    .broadcast_to([P, deg, d]),
                    op=mybir.AluOpType.mult,
                )
                e = epool.tile([P, deg], bf16, tag="e")
                with nc.allow_low_precision(reason="bf16 edge attn within tolerance"):
                    nc.vector.tensor_reduce(
                        out=e[:, :],
                        in_=Pt[:, :].rearrange("p (k f) -> p k f", k=deg),
                        axis=mybir.AxisListType.X,
                        op=mybir.AluOpType.add,
                    )
                Qt = qpool.tile([P, deg * d], bf16, tag="Qt")
                nc.vector.tensor_tensor(
                    out=Qt[:, :].rearrange("p (k f) -> p k f", k=deg),
                    in0=Gv,
                    in1=e[:, :].unsqueeze(2).broadcast_to([P, deg, d]),
                    op=mybir.AluOpType.mult,
                )
                o = opool.tile([P, d], f32, tag="o")
                nc.vector.tensor_reduce(
                    out=o[:, :],
                    in_=Qt[:, :].rearrange("p (k f) -> p f k", k=deg),
                    axis=mybir.AxisListType.X,
                    op=mybir.AluOpType.add,
                )
                nc.sync.dma_start(out=out_ext[t * P : (t + 1) * P, :], in_=o[:, :])

    nc.compile()
    return nc


def make_in_maps(X, weights, attention_w, column_index, n_nodes=N_NODES, cores=CORES):
    npc = n_nodes // cores
    ntiles = (npc + P - 1) // P
    npad = ntiles * P
    s = float(np.asarray(attention_w).reshape(-1)[0])
    w = np.asarray(weights, dtype=np.float32)
    ci_all = np.asarray(column_index, dtype=np.int32).reshape(n_nodes, DEG)
    in_maps = []
    for c in range(cores):
        r0, r1 = c * npc, (c + 1) * npc
        xT = np.zeros((D, npad + 2 * D), dtype=np.float32)
        xT[:, :npc] = np.asarray(X[r0:r1], dtype=np.float32).T
        xT[:, npad : npad + D] = w
        xT[:, npad + D : npad + 2 * D] = w * s
        ci = np.zeros((npad, DEG), dtype=np.int32)
        ci_shard = ci_all[r0:r1]
        # remap node id -> row in the npad-padded AllGather table
        ci[:npc] = (ci_shard // npc) * npad + (ci_shard % npc)
        in_maps.append(
            {
                "xT": np.ascontiguousarray(xT),
                "ci": np.ascontiguousarray(ci),
            }
        )
    return in_maps


_NC_CACHE = {}


def _get_nc():
    key = (N_NODES, NPC)
    if key not in _NC_CACHE:
        _NC_CACHE[key] = build_nc()
    return _NC_CACHE[key]


def run(X, weights, attention_w, column_index, trace=False, **trace_kwargs):
    from concourse import bass_utils

    nc = _get_nc()
    in_maps = make_in_maps(X, weights, attention_w, column_index)
    res = bass_utils.run_bass_kernel_spmd(
        nc, in_maps, core_ids=list(range(CORES)), trace=trace, **trace_kwargs
    )
    outs = [np.asarray(res.results[c]["out"][:NPC]) for c in range(CORES)]
    return np.concatenate(outs, axis=0).astype(np.float32), res


def kernel(
    X,
    weights,
    attention_w,
    row_pointers,
    column_index,
    blockPartition,
    edgeToColumn,
    edgeToRow,
    **_unused,
):
    out, _ = run(X, weights, attention_w, column_index)
    return out


# revision 16
# speedup vs baseline: 2.1072x; 1.1520x over previous
"""AGNNConv (single-head attention message passing) on 8 TRN2 NeuronCores.

Reference computation (N=100000 nodes, fixed degree 16, D=64):
    X_prime = X @ W                                  # [N, 64]
    e[n,k]  = <X_prime[n], X_prime[ci[n,k]]> * s     # s = attention_w[0,0]
    out[n]  = sum_k e[n,k] * X_prime[ci[n,k]]        # [N, 64]

Sharding: nodes split 12500/core across 8 cores. Each core computes its
X_prime shard (and a pre-scaled copy via W*s), AllGathers the bf16 table,
then gathers its nodes' 16 neighbor rows per 128-node tile with an
indirect DMA and does the dot/weight/aggregate on the vector engine.
"""

import math
import sys

import numpy as np

if "/opt/trn_rl_repo" not in sys.path:
    sys.path.insert(0, "/opt/trn_rl_repo")

N_NODES = 100000
DEG = 16
D = 64
CORES = 8
NPC = N_NODES // CORES  # 12500
P = 128
NTILES = (NPC + P - 1) // P  # 98
NPAD = NTILES * P  # 12544


def build_nc(n_nodes=N_NODES, npc=NPC, deg=DEG, d=D, cores=CORES, lowering=False):
    from concourse import bacc, bass, mybir, tile

    ntiles = (npc + P - 1) // P
    npad = ntiles * P

    f32 = mybir.dt.float32
    bf16 = mybir.dt.bfloat16
    i32 = mybir.dt.int32

    nc = bacc.Bacc(
        "TRN2", target_bir_lowering=lowering, debug=False, num_devices=cores
    )

    # xT carries [X_shard.T | W | W*s] so the matmuls depend on ONE input DMA
    # (the Matmult LdWeights slot only fits a single semaphore wait).
    xT = nc.declare_dram_parameter("xT", [d, npad + 2 * d], f32, isOutput=False)
    ci = nc.declare_dram_parameter("ci", [npad, deg], i32, isOutput=False)
    out_ext = nc.declare_dram_parameter("out", [npad, d], f32, isOutput=True)

    # Padded to npad rows so ONE DMA fills cc_in (the collective-trigger
    # ISA struct only fits a single semaphore wait). Neighbor indices are
    # host-remapped to this padded row numbering.
    cc_in = nc.dram_tensor("cc_in", [npad, d], bf16)
    cc_out = nc.dram_tensor("cc_out", [cores * npad, d], bf16, addr_space="Shared")

    with tile.TileContext(nc) as tc:
        with (
            tc.tile_pool(name="const", bufs=1) as cpool,
            tc.tile_pool(name="psum", bufs=4, space="PSUM") as psum,
            tc.tile_pool(name="g", bufs=4) as gpool,
            tc.tile_pool(name="prod", bufs=2) as ppool,
            tc.tile_pool(name="q", bufs=2) as qpool,
            tc.tile_pool(name="e", bufs=3) as epool,
            tc.tile_pool(name="o", bufs=3) as opool,
        ):
            xT_sb = cpool.tile([d, npad + 2 * d], f32, tag="xT_sb")
            xp_bf = cpool.tile([P, ntiles * d], bf16, tag="xp_bf")
            sxp_bf = cpool.tile([P, ntiles * d], bf16, tag="sxp_bf")
            idx_sb = cpool.tile([P, ntiles * deg], i32, tag="idx_sb")

            nc.sync.dma_start(out=xT_sb[:, :], in_=xT[:, :])
            w_sb = xT_sb[:, npad : npad + d]
            w2_sb = xT_sb[:, npad + d : npad + 2 * d]
            nc.sync.dma_start(
                out=idx_sb[:, :].rearrange("p (t k) -> p t k", t=ntiles),
                in_=ci[:, :].rearrange("(t p) k -> p t k", p=P),
            )

            # X_prime shard (bf16) and pre-scaled X_prime*s shard.
            for t in range(ntiles):
                ps1 = psum.tile([P, d], f32, tag="ps1")
                nc.tensor.matmul(
                    ps1[:, :],
                    xT_sb[:, t * P : (t + 1) * P],
                    w_sb,
                    start=True,
                    stop=True,
                )
                nc.vector.tensor_copy(out=xp_bf[:, t * d : (t + 1) * d], in_=ps1[:, :])
                ps2 = psum.tile([P, d], f32, tag="ps2")
                nc.tensor.matmul(
                    ps2[:, :],
                    xT_sb[:, t * P : (t + 1) * P],
                    w2_sb,
                    start=True,
                    stop=True,
                )
                nc.vector.tensor_copy(out=sxp_bf[:, t * d : (t + 1) * d], in_=ps2[:, :])

            # Shard -> internal DRAM -> AllGather full bf16 table.
            nc.sync.dma_start(
                out=cc_in[:, :].rearrange("(t p) f -> p t f", p=P),
                in_=xp_bf[:, :].rearrange("p (t f) -> p t f", t=ntiles),
            )
            nc.gpsimd.collective_compute(
                "AllGather",
                mybir.AluOpType.bypass,
                replica_groups=[list(range(cores))],
                ins=[cc_in.ap()],
                outs=[cc_out.ap()],
            )

            # The SWDGE queue descriptor fits only ONE semaphore wait, but the
            # first gather depends on both the collective (cc_out) and the idx
            # DMA. Absorb each wait into the SWDGE proc with a tiny DMA first.
            scr = cpool.tile([1, d], bf16, tag="scr")
            scr2 = cpool.tile([1, deg], i32, tag="scr2")
            d1 = nc.gpsimd.dma_start(out=scr[:, :], in_=cc_out[0:1, :])
            d2 = nc.gpsimd.dma_start(out=scr2[:, :], in_=idx_sb[0:1, 0:deg])

            # Gather + edge compute, one 128-node tile at a time.
            from concourse.tile import add_dep_helper

            for t in range(ntiles):
                G = gpool.tile([P, deg * d], bf16, tag="G")
                for k in range(deg):
                    # HW DGE consumes ONE offset per partition per instruction:
                    # each descriptor gathers a contiguous row into this
                    # neighbor's slot of G.
                    gi = nc.gpsimd.indirect_dma_start(
                        out=G[:, k * d : (k + 1) * d],
                        out_offset=None,
                        in_=cc_out[:, :],
                        in_offset=bass.IndirectOffsetOnAxis(
                            ap=idx_sb[:, t * deg + k : t * deg + k + 1], axis=0
                        ),
                    )
                    if t < 8:
                        add_dep_helper(gi.ins, d1.ins, False, "swdge wait absorber")
                        add_dep_helper(gi.ins, d2.ins, False, "swdge wait absorber")
                Gv = G[:, :].rearrange("p (k f) -> p k f", k=deg)
                Pt = ppool.tile([P, deg * d], bf16, tag="Pt")
                nc.vector.tensor_tensor(
                    out=Pt[:, :].rearrange("p (k f) -> p k f", k=deg),
                    in0=Gv,
                    in1=sxp_bf[:, t * d : (t + 1) * d]
                    .unsqueeze(1)
                    .broadcast_to([P, deg, d]),
                    op=mybir.AluOpType.mult,
                )
                e = epool.tile([P, deg], bf16, tag="e")
                with nc.allow_low_precision(reason="bf16 edge attn within tolerance"):
                    nc.vector.tensor_reduce(
                        out=e[:, :],
                        in_=Pt[:, :].rearrange("p (k f) -> p k f", k=deg),
                        axis=mybir.AxisListType.X,
                        op=mybir.AluOpType.add,
                    )
                Qt = qpool.tile([P, deg * d], bf16, tag="Qt")
                nc.vector.tensor_tensor(
                    out=Qt[:, :].rearrange("p (k f) -> p k f", k=deg),
                    in0=Gv,
                    in1=e[:, :].unsqueeze(2).broadcast_to([P, deg, d]),
                    op=mybir.AluOpType.mult,
                )
                o = opool.tile([P, d], f32, tag="o")
                nc.vector.tensor_reduce(
                    out=o[:, :],
                    in_=Qt[:, :].rearrange("p (k f) -> p f k", k=deg),
                    axis=mybir.AxisListType.X,
                    op=mybir.AluOpType.add,
                )
                nc.sync.dma_start(out=out_ext[t * P : (t + 1) * P, :], in_=o[:, :])

    nc.compile()
    return nc


def make_in_maps(X, weights, attention_w, column_index, n_nodes=N_NODES, cores=CORES):
    npc = n_nodes // cores
    ntiles = (npc + P - 1) // P
    npad = ntiles * P
    s = float(np.asarray(attention_w).reshape(-1)[0])
    w = np.asarray(weights, dtype=np.float32)
    ci_all = np.asarray(column_index, dtype=np.int32).reshape(n_nodes, DEG)
    in_maps = []
    for c in range(cores):
        r0, r1 = c * npc, (c + 1) * npc
        xT = np.zeros((D, npad + 2 * D), dtype=np.float32)
        xT[:, :npc] = np.asarray(X[r0:r1], dtype=np.float32).T
        xT[:, npad : npad + D] = w
        xT[:, npad + D : npad + 2 * D] = w * s
        ci = np.zeros((npad, DEG), dtype=np.int32)
        ci_shard = ci_all[r0:r1]
        # remap node id -> row in the npad-padded AllGather table
        ci[:npc] = (ci_shard // npc) * npad + (ci_shard % npc)
        in_maps.append(
            {
                "xT": np.ascontiguousarray(xT),
                "ci": np.ascontiguousarray(ci),
            }
        )
    return in_maps


_NC_CACHE = {}


def _get_nc():
    key = (N_NODES, NPC)
    if key not in _NC_CACHE:
        _NC_CACHE[key] = build_nc()
    return _NC_CACHE[key]


def run(X, weights, attention_w, column_index, trace=False, **trace_kwargs):
    from concourse import bass_utils

    nc = _get_nc()
    in_maps = make_in_maps(X, weights, attention_w, column_index)
    res = bass_utils.run_bass_kernel_spmd(
        nc, in_maps, core_ids=list(range(CORES)), trace=trace, **trace_kwargs
    )
    outs = [np.asarray(res.results[c]["out"][:NPC]) for c in range(CORES)]
    return np.concatenate(outs, axis=0).astype(np.float32), res


def kernel(
    X,
    weights,
    attention_w,
    row_pointers,
    column_index,
    blockPartition,
    edgeToColumn,
    edgeToRow,
    **_unused,
):
    out, _ = run(X, weights, attention_w, column_index)
    return out
